# revision 1
# baseline (speedup 1.0000x reference)
"""Multi-head causal self-attention on 8 Trainium2 NeuronCores.

Problem: B=8, T=1024, D=1024, H=16 heads, DH=64.
    q,k,v = einsum('btd,hdk->bhtk', x, W{q,k,v})
    scores = q @ k.T / sqrt(DH), causal mask, softmax
    out = (softmax @ v) reshaped -> [B,T,H*DH] @ Wo + bo

Sharding: batch-parallel, one batch element per core (B == n_cores == 8).
No collectives needed; weights are replicated to every core.

Per-core dataflow (transpose-free):
  xT [d,t] (host-transposed) and W matrices live with d on partitions, so
  QT/KT come out as [dh, t] (heads pair-packed to fill 128 partitions) and
  V as [t, dh] (heads quad-packed for free-dim >=256, which fp32r needs to
  run at 1 cycle/row).  Scores are computed transposed, ST[s,q] = KT.T@QT,
  so no P transpose is needed for the AV matmul: OT[dh,q] = (V|1).T @ exp(ST).
  A ones column appended to V yields the softmax denominator for free in
  row 64 of the AV psum.  exp() is applied without max-subtraction (scores
  are O(5) for randn inputs; exp stays well inside fp32 range) and causal
  masking zeroes exp(S) after the fact, so no -inf handling is needed.
  Normalization divides OT by the broadcast denominator, and the Wo
  projection consumes OT directly as the stationary operand, producing
  final[q,d] which DMAs out contiguously.

All matmuls run in float32r (TF32-like, ~1.2e-4 rel err, 4x faster than
fp32).  This walrus build only allows ONE sync-wait per instruction, so a
post-scheduling pass hoists extra waits onto inserted PE no-ops.
"""

import sys

for _p in ("/opt/trn_rl_repo", "/root/.axon_site/_ro/trn_rl_repo"):
    if _p not in sys.path:
        sys.path.insert(0, _p)

import numpy as np

import concourse.bass as bass
import concourse.mybir as mybir
import concourse.tile as tile

f32 = mybir.dt.float32
f32r = mybir.dt.float32r

B, T, D, H, DH = 8, 1024, 1024, 16, 64
NP = 128            # partitions
NC = 512            # matmul free-dim chunk (fp32 moving-operand max)
KT_ = D // NP       # 8 contraction tiles over d
NT = T // NP        # 8 tiles over t (s and q tiles)
NCH = T // NC       # 2 free-dim chunks over q
NPAIR = H // 2      # 8 head pairs   (QT/KT pack 2 heads on partitions)
NQUAD = H // 4      # 4 head quads   (V packs 4 heads on free dim)


def build_nc(split_waits=True):
    nc = bass.Bass(trn_type="TRN2")
    xt = nc.dram_tensor("xt", [D, T], f32r, kind="ExternalInput")
    wq = nc.dram_tensor("wq", [D, H * DH], f32r, kind="ExternalInput")
    wk = nc.dram_tensor("wk", [D, H * DH], f32r, kind="ExternalInput")
    wv = nc.dram_tensor("wv", [D, H * DH], f32r, kind="ExternalInput")
    wo = nc.dram_tensor("wo", [H * DH, D], f32r, kind="ExternalInput")
    bo = nc.dram_tensor("bo", [1, D], f32, kind="ExternalInput")
    out = nc.dram_tensor("out", [T, D], f32, kind="ExternalOutput")

    with tile.TileContext(nc) as tc:
        _mha(tc, nc, xt, wq, wk, wv, wo, bo, out)

    if split_waits:
        _split_waits(nc)
    return nc


def _mha(tc, nc, xt, wq, wk, wv, wo, bo, out):
    import contextlib

    ctx = contextlib.ExitStack()
    singles = ctx.enter_context(tc.tile_pool(name="singles", bufs=1))
    bigpool = ctx.enter_context(tc.tile_pool(name="bigpool", bufs=1))
    wpool = ctx.enter_context(tc.tile_pool(name="wpool", bufs=1))
    qkpool = ctx.enter_context(tc.tile_pool(name="qkpool", bufs=2))
    vpool = ctx.enter_context(tc.tile_pool(name="vpool", bufs=2))
    pexpool = ctx.enter_context(tc.tile_pool(name="pexpool", bufs=7))
    avpool = ctx.enter_context(tc.tile_pool(name="avpool", bufs=8))
    smalls = ctx.enter_context(tc.tile_pool(name="smalls", bufs=9))
    bcpool = ctx.enter_context(tc.tile_pool(name="bcpool", bufs=2))
    fpool = ctx.enter_context(tc.tile_pool(name="fpool", bufs=2))
    ps_proj = ctx.enter_context(tc.tile_pool(name="ps_proj", bufs=2, space="PSUM"))
    ps_st = ctx.enter_context(tc.tile_pool(name="ps_st", bufs=4, space="PSUM"))
    ps_av = ctx.enter_context(tc.tile_pool(name="ps_av", bufs=2, space="PSUM"))

    def act_recip(out_ap, in_ap):
        """ACT-engine reciprocal via raw InstActivation (nc.scalar.activation
        refuses Reciprocal; measured 1.5e-6 rel err on our denominator range,
        and 4.6x cheaper than the single-lane DVE reciprocal)."""
        ins = [nc.scalar.lower_ap(in_ap)]
        for arg in (0.0, 1.0, 0.0):                     # bias, scale, alpha
            ins.append(mybir.ImmediateValue(dtype=f32, value=arg))
        nc.scalar.add_instruction(mybir.InstActivation(
            name=nc.get_next_instruction_name(),
            func=mybir.ActivationFunctionType.Reciprocal,
            ins=ins,
            outs=[nc.scalar.lower_ap(out_ap)],
        ))

    with ctx:
        # --- resident inputs -------------------------------------------------
        # (memset can't write f32r directly; memset f32 then round via copy)
        onesf = singles.tile([NP, 1], f32)
        nc.vector.memset(onesf, 1.0)
        ones_row = singles.tile([1, DH], f32r)           # K=1 bcast matmul lhsT
        nc.vector.tensor_copy(out=ones_row, in_=onesf[0:1, 0:1].to_broadcast((1, DH)))
        # x^T and Wo share one 4MB slot: Wo is only needed after the last
        # QKV projection has consumed x^T
        xt_sb = bigpool.tile([NP, KT_, T], f32r, tag="big", name="xt_sb")
        nc.sync.dma_start(out=xt_sb, in_=xt.rearrange("(kt p) t -> p kt t", p=NP))
        bo_bc = singles.tile([NP, D], f32)               # bias broadcast to rows
        nc.sync.dma_start(out=bo_bc, in_=bo[0:1, :].to_broadcast((NP, D)))

        # out^T accumulator for all heads: [dh(pair-packed), pair, q]
        ot_sb = singles.tile([NP, NPAIR, T], f32r)

        # deferred normalization: (avsb, den_sb, pair, hh, c) per head-chunk;
        # flushed inside the NEXT quad's projection phase (PE never waits on
        # the reciprocal chain, and ACT batches recips = 2 table switches)
        norm_pending = []

        def flush_normalizes():
            items = list(norm_pending)
            norm_pending.clear()
            recips = []
            for avsb, den_sb, pair, hh, c in items:
                recip_sb = smalls.tile([1, NC], f32r, tag="recip", name="recip_sb")
                act_recip(recip_sb, den_sb)
                recips.append(recip_sb)
            for (avsb, den_sb, pair, hh, c), recip_sb in zip(items, recips):
                bc_ps = ps_st.tile([DH, NC], f32, tag="st_ps", name="bc_ps")
                nc.tensor.matmul(
                    out=bc_ps, lhsT=ones_row, rhs=recip_sb,
                    start=True, stop=True)
                bcast = bcpool.tile([DH, NC], f32, tag="bcast", name="bcast")
                nc.vector.tensor_copy(out=bcast, in_=bc_ps)
                nc.vector.tensor_mul(
                    out=ot_sb[hh * DH:(hh + 1) * DH, pair, c * NC:(c + 1) * NC],
                    in0=avsb,
                    in1=bcast,
                )

        wo_sb_holder = []

        for quad in range(NQUAD):
            cs = quad * 4 * DH                          # column start in w mats
            wq_sb = wpool.tile([NP, KT_, 4 * DH], f32r, tag="wq")
            wk_sb = wpool.tile([NP, KT_, 4 * DH], f32r, tag="wk")
            wv_sb = wpool.tile([NP, KT_, 4 * DH], f32r, tag="wv")
            nc.sync.dma_start(
                out=wq_sb, in_=wq[:, cs:cs + 4 * DH].rearrange("(kt p) c -> p kt c", p=NP))
            nc.sync.dma_start(
                out=wk_sb, in_=wk[:, cs:cs + 4 * DH].rearrange("(kt p) c -> p kt c", p=NP))
            nc.sync.dma_start(
                out=wv_sb, in_=wv[:, cs:cs + 4 * DH].rearrange("(kt p) c -> p kt c", p=NP))

            # --- QT / KT projections: [2*DH(partitions), T] per head pair ----
            qk_tiles = {}
            for name, w_sb in (("q", wq_sb), ("k", wk_sb)):
                for pp in range(2):                      # pair within quad
                    t_sb = qkpool.tile([NP, T], f32r, tag=f"{name}t", name=f"{name}t_sb")
                    for c in range(NCH):
                        psum = ps_proj.tile([NP, NC], f32, name="proj_ps")
                        for kd in range(KT_):
                            nc.tensor.matmul(
                                out=psum,
                                lhsT=w_sb[:, kd, pp * NP:(pp + 1) * NP],
                                rhs=xt_sb[:, kd, c * NC:(c + 1) * NC],
                                start=(kd == 0), stop=(kd == KT_ - 1),
                            )
                        nc.vector.tensor_copy(out=t_sb[:, c * NC:(c + 1) * NC], in_=psum)
                    qk_tiles[(name, pp)] = t_sb

            # previous quad's softmax normalizations run here, hidden under
            # the projection matmul stream
            if norm_pending:
                flush_normalizes()

            # --- V (+ones col): [t(partitions), head, s-tile, DH+1] ----------
            v1_sb = vpool.tile([NP, 4, NT, DH + 1], f32r)
            nc.vector.tensor_copy(
                out=v1_sb[:, :, :, DH:DH + 1],
                in_=onesf.to_broadcast((NP, 4, NT, 1)))
            for tt in range(NT):
                psum = ps_proj.tile([NP, 4 * DH], f32, name="vproj_ps", tag="proj_ps")
                for kd in range(KT_):
                    nc.tensor.matmul(
                        out=psum,
                        lhsT=xt_sb[:, kd, tt * NP:(tt + 1) * NP],
                        rhs=wv_sb[:, kd, :],
                        start=(kd == 0), stop=(kd == KT_ - 1),
                    )
                for h in range(4):
                    nc.vector.tensor_copy(
                        out=v1_sb[:, h, tt, 0:DH], in_=psum[:, h * DH:(h + 1) * DH])

            if quad == NQUAD - 1:
                # Wo reuses x^T's slot (x^T fully consumed by the V matmuls
                # above); the 4MB DMA overlaps this quad's attention phase
                wo_sb = bigpool.tile([NP, KT_, D], f32r, tag="big", name="wo_sb")
                nc.sync.dma_start(
                    out=wo_sb, in_=wo.rearrange("(kt p) d -> p kt d", p=NP))
                wo_sb_holder.append(wo_sb)

            # --- attention: scores+exp+AV pipelined at the s-tile level ------
            # diagonal blocks only compute their live columns (causal trim);
            # AV matmuls for s-tile j-1 are emitted after the score matmuls
            # for s-tile j so PE overlaps ACT's exp / GPSIMD's mask-select
            for pp in range(2):
                pair = quad * 2 + pp
                qt = qk_tiles[("q", pp)]
                kt = qk_tiles[("k", pp)]
                for c in range(NCH):
                    jmax = 4 * c + 4                    # causal: s-tiles 0..jmax-1
                    av = [ps_av.tile([NP, NC], f32, name="av_ps", tag="av_ps")
                          for _ in range(2)]

                    def _emit_st(j):
                        co = min(max(0, j - 4 * c) * NP, NC - 256)  # col trim
                        st_ps = []
                        for hh in range(2):             # head within pair
                            hp = hh * DH                # partition offset (0|64)
                            st_psum = ps_st.tile([NP, NC], f32, name="st_ps")
                            nc.tensor.matmul(
                                out=st_psum[:, co:NC],
                                lhsT=kt[hp:hp + DH, j * NP:(j + 1) * NP],
                                rhs=qt[hp:hp + DH, c * NC + co:(c + 1) * NC],
                                start=True, stop=True,
                            )
                            st_ps.append(st_psum)
                        outp = []
                        for hh in range(2):
                            p_sb = pexpool.tile([NP, NC], f32r, name="p_sb")
                            nc.scalar.activation(
                                out=p_sb[:, co:NC], in_=st_ps[hh][:, co:NC],
                                func=mybir.ActivationFunctionType.Exp)
                            if j >= 4 * c:              # diagonal block: mask
                                nc.gpsimd.affine_select(
                                    out=p_sb[:, co:NC], in_=p_sb[:, co:NC],
                                    pattern=[[1, NC - co]],
                                    compare_op=mybir.AluOpType.is_ge,
                                    fill=0.0,
                                    base=c * NC + co - j * NP,
                                    channel_multiplier=-1,
                                )
                            outp.append(p_sb)
                        return co, outp

                    def _emit_av(j, co, pexp_j):
                        for hh in range(2):
                            h = 2 * pp + hh             # head within quad
                            nc.tensor.matmul(
                                out=av[hh][0:DH + 1, co:NC],
                                lhsT=v1_sb[:, h, j, :],
                                rhs=pexp_j[hh][:, co:NC],
                                start=(j == 0), stop=(j == jmax - 1),
                                skip_group_check=True,
                            )

                    prev = None
                    for j in range(jmax):
                        cur = (j,) + _emit_st(j)
                        if prev is not None:
                            _emit_av(*prev)
                        prev = cur
                    _emit_av(*prev)

                    for hh in range(2):
                        avsb = avpool.tile([DH, NC], f32, name="avsb")
                        nc.vector.tensor_copy(out=avsb, in_=av[hh][0:DH, :])
                        den_sb = smalls.tile([1, NC], f32r, tag="den", name="den_sb")
                        nc.vector.tensor_copy(out=den_sb, in_=av[hh][DH:DH + 1, :])
                        norm_pending.append((avsb, den_sb, pair, hh, c))

        flush_normalizes()                              # last quad's items
        wo_sb = wo_sb_holder[0]

        # --- Wo projection: final[q, d] = sum_pair OT.T @ Wo + bo ------------
        for qi in range(NT):
            f_sb = fpool.tile([NP, D], f32, name="f_sb")
            for dc in range(NCH):
                wo_ps = ps_av.tile([NP, NC], f32, tag="av_ps", name="wo_ps")
                for pp in range(NPAIR):
                    nc.tensor.matmul(
                        out=wo_ps,
                        lhsT=ot_sb[:, pp, qi * NP:(qi + 1) * NP],
                        rhs=wo_sb[:, pp, dc * NC:(dc + 1) * NC],
                        start=(pp == 0), stop=(pp == NPAIR - 1),
                    )
                nc.vector.tensor_add(
                    out=f_sb[:, dc * NC:(dc + 1) * NC],
                    in0=wo_ps,
                    in1=bo_bc[:, dc * NC:(dc + 1) * NC],
                )
            nc.sync.dma_start(out=out[qi * NP:(qi + 1) * NP, :], in_=f_sb)


def _split_waits(nc, max_waits=1):
    """Walrus on this target allows one sync-wait per instruction; hoist
    extras onto no-ops inserted just before the offending instruction."""
    for f in nc.m.functions:
        for b in f.blocks:
            insts = b.instructions
            new = []
            changed = False
            for inst in insts:
                si = inst.sync_info
                if si is not None and len(si.on_wait) > max_waits:
                    waits = list(si.on_wait)
                    extra, keep = waits[:-max_waits], waits[-max_waits:]
                    for j, w in enumerate(extra):
                        new.append(mybir.InstNoOp(
                            name=f"{inst.name}-wnop{j}",
                            sync_info=mybir.SyncInfo(on_wait=[w], on_update=[]),
                            engine=inst.engine,
                            bass_nofuse=True,
                        ))
                    inst.sync_info = mybir.SyncInfo(
                        on_wait=keep, on_update=list(si.on_update))
                    changed = True
                new.append(inst)
            if changed:
                b.instructions = new


def make_in_maps(x, Wq, Wk, Wv, Wo, bo):
    scale = np.float32(DH) ** np.float32(-0.5)
    # [H, D, DH] -> [D, H*DH]; fold the 1/sqrt(DH) score scale into Wq
    wq_m = np.ascontiguousarray(
        Wq.transpose(1, 0, 2).reshape(D, H * DH) * scale).astype(np.float32)
    wk_m = np.ascontiguousarray(Wk.transpose(1, 0, 2).reshape(D, H * DH)).astype(np.float32)
    wv_m = np.ascontiguousarray(Wv.transpose(1, 0, 2).reshape(D, H * DH)).astype(np.float32)
    wo_m = np.ascontiguousarray(Wo).astype(np.float32)
    bo_m = np.ascontiguousarray(bo.reshape(1, D)).astype(np.float32)
    return [
        {
            "xt": np.ascontiguousarray(np.asarray(x[b]).T).astype(np.float32),
            "wq": wq_m, "wk": wk_m, "wv": wv_m, "wo": wo_m, "bo": bo_m,
        }
        for b in range(B)
    ]


_NC_CACHE = []


def kernel(x, Wq, Wk, Wv, Wo, bo):
    from concourse.bass_utils import run_bass_kernel_spmd

    x = np.asarray(x)
    if not _NC_CACHE:
        _NC_CACHE.append(build_nc())
    nc = _NC_CACHE[0]
    in_maps = make_in_maps(x, np.asarray(Wq), np.asarray(Wk), np.asarray(Wv),
                           np.asarray(Wo), np.asarray(bo))
    res = run_bass_kernel_spmd(nc, in_maps, core_ids=list(range(B)))
    return np.stack([res.results[b]["out"] for b in range(B)]).astype(np.float32)



# revision 8
# speedup vs baseline: 1.4476x; 1.4476x over previous
"""Multi-head causal self-attention on 8 Trainium2 NeuronCores.

Problem: B=8, T=1024, D=1024, H=16 heads, DH=64.
    q,k,v = einsum('btd,hdk->bhtk', x, W{q,k,v})
    scores = q @ k.T / sqrt(DH), causal mask, softmax
    out = (softmax @ v) reshaped -> [B,T,H*DH] @ Wo + bo

Sharding: batch-parallel, one batch element per core (B == n_cores == 8).
No collectives; weights replicated to every core.

Per-core dataflow (transpose-free), v2 = bf16 streams + merged ACT work:
  All matmul operands are bf16 (PSUM accumulation stays fp32), which keeps
  the PE at its 1 row/cycle stream rate and halves SBUF/DMA traffic; the
  softmax denominator path stays fp32.
  xT [d,t] (host-transposed) lives with d on partitions; QT/KT come out as
  [dh, t] (heads pair-packed on partitions) and V as [t, dh] (heads
  quad-packed).  Scores are computed transposed, ST[s,q] = KT.T@QT, into a
  single 2-bank PSUM pair tile [128, 2(head), 512] so ONE ACT exp and ONE
  GpSimd affine-select cover both heads of the pair (halves the per-j-step
  instruction overhead on the two engines that pace the attention inner
  loop).  A ones column appended to V yields the softmax denominator in row
  64 of the AV psum.  exp() is applied without max-subtraction (scores are
  O(5) for randn inputs) and causal masking zeroes exp(S) after the fact.
  Normalization is deferred and flushed during the NEXT quad's projection
  phase: per (pair, chunk) the two heads' denominators are packed to a
  [2, 512] tile (DVE copies may shift partitions), reciprocal'd in one ACT
  instruction, broadcast to 128 partitions with a single K=2 selector
  matmul, and applied with one DVE multiply.  The Wo projection consumes
  OT directly as the stationary operand, producing final[q,d] which DMAs
  out contiguously.

This walrus build only allows ONE sync-wait per instruction, so a
post-scheduling pass hoists extra waits onto inserted PE no-ops.
"""

import sys

for _p in ("/opt/trn_rl_repo", "/root/.axon_site/_ro/trn_rl_repo"):
    if _p not in sys.path:
        sys.path.insert(0, _p)

import numpy as np

import concourse.bass as bass
import concourse.mybir as mybir
import concourse.tile as tile

f32 = mybir.dt.float32
f32r = mybir.dt.float32r
bf16 = mybir.dt.bfloat16

B, T, D, H, DH = 8, 1024, 1024, 16, 64
NP = 128            # partitions
NC = 512            # matmul free-dim chunk (PSUM bank = 512 fp32)
KT_ = D // NP       # 8 contraction tiles over d
NT = T // NP        # 8 tiles over t (s and q tiles)
NCH = T // NC       # 2 free-dim chunks over q
NPAIR = H // 2      # 8 head pairs   (QT/KT pack 2 heads on partitions)
NQUAD = H // 4      # 4 head quads   (V packs 4 heads on free dim)


def build_nc(split_waits=True):
    nc = bass.Bass(trn_type="TRN2")
    xt = nc.dram_tensor("xt", [D, T], bf16, kind="ExternalInput")
    wq = nc.dram_tensor("wq", [D, H * DH], bf16, kind="ExternalInput")
    wk = nc.dram_tensor("wk", [D, H * DH], bf16, kind="ExternalInput")
    wv = nc.dram_tensor("wv", [D, H * DH], bf16, kind="ExternalInput")
    wo = nc.dram_tensor("wo", [H * DH, D], bf16, kind="ExternalInput")
    bo = nc.dram_tensor("bo", [1, D], f32, kind="ExternalInput")
    out = nc.dram_tensor("out", [T, D], f32, kind="ExternalOutput")

    with tile.TileContext(nc) as tc:
        _mha(tc, nc, xt, wq, wk, wv, wo, bo, out)

    if split_waits:
        _split_waits(nc)
    return nc


def _mha(tc, nc, xt, wq, wk, wv, wo, bo, out):
    import contextlib

    ctx = contextlib.ExitStack()
    singles = ctx.enter_context(tc.tile_pool(name="singles", bufs=1))
    bigpool = ctx.enter_context(tc.tile_pool(name="bigpool", bufs=1))
    wpool = ctx.enter_context(tc.tile_pool(name="wpool", bufs=2))
    qkpool = ctx.enter_context(tc.tile_pool(name="qkpool", bufs=4))
    vpool = ctx.enter_context(tc.tile_pool(name="vpool", bufs=2))
    pexpool = ctx.enter_context(tc.tile_pool(name="pexpool", bufs=4))
    avpool = ctx.enter_context(tc.tile_pool(name="avpool", bufs=5))
    denpool = ctx.enter_context(tc.tile_pool(name="denpool", bufs=5))
    recpool = ctx.enter_context(tc.tile_pool(name="recpool", bufs=5))
    fpool = ctx.enter_context(tc.tile_pool(name="fpool", bufs=2))
    ps_proj = ctx.enter_context(tc.tile_pool(name="ps_proj", bufs=2, space="PSUM"))
    ps_st = ctx.enter_context(tc.tile_pool(name="ps_st", bufs=2, space="PSUM"))
    ps_av = ctx.enter_context(tc.tile_pool(name="ps_av", bufs=2, space="PSUM"))

    def act_recip(out_ap, in_ap):
        """ACT-engine reciprocal via raw InstActivation (nc.scalar.activation
        refuses Reciprocal; ~1.5e-6 rel err on our denominator range)."""
        ins = [nc.scalar.lower_ap(in_ap)]
        for arg in (0.0, 1.0, 0.0):                     # bias, scale, alpha
            ins.append(mybir.ImmediateValue(dtype=f32, value=arg))
        nc.scalar.add_instruction(mybir.InstActivation(
            name=nc.get_next_instruction_name(),
            func=mybir.ActivationFunctionType.Reciprocal,
            ins=ins,
            outs=[nc.scalar.lower_ap(out_ap)],
        ))

    with ctx:
        # --- resident constants ---------------------------------------------
        onesf = singles.tile([NP, 1], f32)
        nc.vector.memset(onesf, 1.0)
        ones_row = singles.tile([1, DH], f32r)           # K=1 bcast matmul lhsT
        nc.vector.tensor_copy(out=ones_row, in_=onesf[0:1, 0:1].to_broadcast((1, DH)))

        # x^T and Wo share one 2MB slot: Wo is only needed after the last
        # QKV projection has consumed x^T
        xt_sb = bigpool.tile([NP, KT_, T], bf16, tag="big", name="xt_sb")
        xtr = xt.rearrange("(kt p) t -> p kt t", p=NP)
        nc.sync.dma_start(out=xt_sb[:, :, 0:NC], in_=xtr[:, :, 0:NC])
        nc.sync.dma_start(out=xt_sb[:, :, NC:T], in_=xtr[:, :, NC:T])
        bo_bc = singles.tile([NP, D], f32)               # bias broadcast to rows
        nc.sync.dma_start(out=bo_bc, in_=bo[0:1, :].to_broadcast((NP, D)))

        # out^T accumulator for all heads: [dh(pair-packed), pair, q]
        ot_sb = singles.tile([NP, NPAIR, T], bf16)

        # deferred normalization: (avsb, den, pair, c) per pair-chunk;
        # flushed inside the NEXT quad's projection phase
        norm_pending = []

        def flush_normalizes():
            items = list(norm_pending)
            norm_pending.clear()
            recs = []
            for avsb, den, pair, c in items:
                # both heads' denominators sit side-by-side on partition 0:
                # one ACT reciprocal covers the pair
                rec = recpool.tile([1, 2, NC], f32r, tag="rec", name="rec")
                act_recip(rec, den)
                recs.append(rec)
            for (avsb, den, pair, c), rec in zip(items, recs):
                for hh in range(2):
                    bc_ps = ps_proj.tile([DH, NC], f32, tag="proj_ps", name="bc_ps")
                    nc.tensor.matmul(
                        out=bc_ps, lhsT=ones_row,
                        rhs=rec[0:1, hh, :], start=True, stop=True)
                    nc.vector.tensor_mul(
                        out=ot_sb[hh * DH:(hh + 1) * DH, pair, c * NC:(c + 1) * NC],
                        in0=avsb[hh * DH:(hh + 1) * DH, :],
                        in1=bc_ps,
                    )

        wo_sb_holder = []

        for quad in range(NQUAD):
            cs = quad * 4 * DH                          # column start in w mats
            wq_sb = wpool.tile([NP, KT_, 4 * DH], bf16, tag="wq")
            wk_sb = wpool.tile([NP, KT_, 4 * DH], bf16, tag="wk")
            wv_sb = wpool.tile([NP, KT_, 4 * DH], bf16, tag="wv")
            nc.sync.dma_start(
                out=wq_sb, in_=wq[:, cs:cs + 4 * DH].rearrange("(kt p) c -> p kt c", p=NP))
            nc.sync.dma_start(
                out=wk_sb, in_=wk[:, cs:cs + 4 * DH].rearrange("(kt p) c -> p kt c", p=NP))
            nc.sync.dma_start(
                out=wv_sb, in_=wv[:, cs:cs + 4 * DH].rearrange("(kt p) c -> p kt c", p=NP))

            # --- QT / KT projections: [2*DH(partitions), T] per head pair ----
            qk_tiles = {}
            for name, w_sb in (("q", wq_sb), ("k", wk_sb)):
                for pp in range(2):                      # pair within quad
                    t_sb = qkpool.tile([NP, T], bf16, tag=f"{name}t", name=f"{name}t_sb")
                    for c in range(NCH):
                        psum = ps_proj.tile([NP, NC], f32, name="proj_ps")
                        for kd in range(KT_):
                            nc.tensor.matmul(
                                out=psum,
                                lhsT=w_sb[:, kd, pp * NP:(pp + 1) * NP],
                                rhs=xt_sb[:, kd, c * NC:(c + 1) * NC],
                                start=(kd == 0), stop=(kd == KT_ - 1),
                            )
                        nc.vector.tensor_copy(out=t_sb[:, c * NC:(c + 1) * NC], in_=psum)
                    qk_tiles[(name, pp)] = t_sb

            # previous quad's softmax normalizations run here, hidden under
            # the projection matmul stream
            if norm_pending:
                flush_normalizes()

            # --- V (+ones col): [t(partitions), head, s-tile, DH+1] ----------
            v1_sb = vpool.tile([NP, 4, NT, DH + 1], bf16)
            nc.vector.tensor_copy(
                out=v1_sb[:, :, :, DH:DH + 1],
                in_=onesf.to_broadcast((NP, 4, NT, 1)))
            for tt in range(NT):
                psum = ps_proj.tile([NP, 4 * DH], f32, name="vproj_ps", tag="proj_ps")
                for kd in range(KT_):
                    nc.tensor.matmul(
                        out=psum,
                        lhsT=xt_sb[:, kd, tt * NP:(tt + 1) * NP],
                        rhs=wv_sb[:, kd, :],
                        start=(kd == 0), stop=(kd == KT_ - 1),
                    )
                for h in range(4):
                    nc.vector.tensor_copy(
                        out=v1_sb[:, h, tt, 0:DH], in_=psum[:, h * DH:(h + 1) * DH])

            if quad == NQUAD - 1:
                # Wo reuses x^T's slot (x^T fully consumed by the V matmuls
                # above); the DMA overlaps this quad's attention phase
                wo_sb = bigpool.tile([NP, KT_, D], bf16, tag="big", name="wo_sb")
                nc.sync.dma_start(
                    out=wo_sb, in_=wo.rearrange("(kt p) d -> p kt d", p=NP))
                wo_sb_holder.append(wo_sb)

            # --- attention: scores+exp+AV pipelined at the s-tile level ------
            # diagonal blocks only compute their live columns (causal trim);
            # AV matmuls for s-tile j-1 are emitted after the score matmuls
            # for s-tile j so PE overlaps ACT's exp / GpSimd's mask-select
            for pp in range(2):
                pair = quad * 2 + pp
                qt = qk_tiles[("q", pp)]
                kt = qk_tiles[("k", pp)]
                for c in range(NCH):
                    jmax = 4 * c + 4                    # causal: s-tiles 0..jmax-1
                    av = [ps_av.tile([DH + 1, NC], f32, name="av_ps", tag="av_ps")
                          for _ in range(2)]

                    def _emit_st(j):
                        co = min(max(0, j - 4 * c) * NP, NC - NP)   # col trim
                        stp = ps_st.tile([NP, 2, NC], f32, name="st_ps", tag="st_ps")
                        for hh in range(2):             # head within pair
                            hp = hh * DH                # partition offset (0|64)
                            nc.tensor.matmul(
                                out=stp[:, hh, co:NC],
                                lhsT=kt[hp:hp + DH, j * NP:(j + 1) * NP],
                                rhs=qt[hp:hp + DH, c * NC + co:(c + 1) * NC],
                                start=True, stop=True,
                            )
                        p_sb = pexpool.tile([NP, 2, NC], bf16, name="p_sb")
                        nc.scalar.activation(
                            out=p_sb[:, :, co:NC], in_=stp[:, :, co:NC],
                            func=mybir.ActivationFunctionType.Exp)
                        if j >= 4 * c:                  # diagonal block: mask
                            nc.gpsimd.affine_select(
                                out=p_sb[:, :, co:NC], in_=p_sb[:, :, co:NC],
                                pattern=[[0, 2], [1, NC - co]],
                                compare_op=mybir.AluOpType.is_ge,
                                fill=0.0,
                                base=c * NC + co - j * NP,
                                channel_multiplier=-1,
                            )
                        return co, p_sb

                    def _emit_av(j, co, p_sb):
                        for hh in range(2):
                            h = 2 * pp + hh             # head within quad
                            nc.tensor.matmul(
                                out=av[hh][0:DH + 1, co:NC],
                                lhsT=v1_sb[:, h, j, :],
                                rhs=p_sb[:, hh, co:NC],
                                start=(j == 0), stop=(j == jmax - 1),
                                skip_group_check=True,
                            )

                    prev = None
                    for j in range(jmax):
                        cur = (j,) + _emit_st(j)
                        if prev is not None:
                            _emit_av(*prev)
                        prev = cur
                    _emit_av(*prev)

                    # drain: AV outputs + denominators (row 64), pair-packed
                    avsb = avpool.tile([NP, NC], f32, name="avsb")
                    den = denpool.tile([1, 2, NC], f32, tag="den", name="den")
                    for hh in range(2):
                        nc.vector.tensor_copy(
                            out=avsb[hh * DH:(hh + 1) * DH, :],
                            in_=av[hh][0:DH, :])
                        nc.vector.tensor_copy(
                            out=den[0:1, hh, :], in_=av[hh][DH:DH + 1, :])
                    norm_pending.append((avsb, den, pair, c))

        flush_normalizes()                              # last quad's items
        wo_sb = wo_sb_holder[0]

        # --- Wo projection: final[q, d] = sum_pair OT.T @ Wo + bo ------------
        for qi in range(NT):
            f_sb = fpool.tile([NP, D], f32, name="f_sb")
            for dc in range(NCH):
                wo_ps = ps_av.tile([NP, NC], f32, tag="av_ps", name="wo_ps")
                for pp in range(NPAIR):
                    nc.tensor.matmul(
                        out=wo_ps,
                        lhsT=ot_sb[:, pp, qi * NP:(qi + 1) * NP],
                        rhs=wo_sb[:, pp, dc * NC:(dc + 1) * NC],
                        start=(pp == 0), stop=(pp == NPAIR - 1),
                    )
                nc.vector.tensor_add(
                    out=f_sb[:, dc * NC:(dc + 1) * NC],
                    in0=wo_ps,
                    in1=bo_bc[:, dc * NC:(dc + 1) * NC],
                )
            nc.sync.dma_start(out=out[qi * NP:(qi + 1) * NP, :], in_=f_sb)


def _split_waits(nc, max_waits=1):
    """Walrus on this target allows one sync-wait per instruction; hoist
    extras onto no-ops inserted just before the offending instruction."""
    for f in nc.m.functions:
        for b in f.blocks:
            insts = b.instructions
            new = []
            changed = False
            for inst in insts:
                si = inst.sync_info
                if si is not None and len(si.on_wait) > max_waits:
                    waits = list(si.on_wait)
                    extra, keep = waits[:-max_waits], waits[-max_waits:]
                    for j, w in enumerate(extra):
                        new.append(mybir.InstNoOp(
                            name=f"{inst.name}-wnop{j}",
                            sync_info=mybir.SyncInfo(on_wait=[w], on_update=[]),
                            engine=inst.engine,
                            bass_nofuse=True,
                        ))
                    inst.sync_info = mybir.SyncInfo(
                        on_wait=keep, on_update=list(si.on_update))
                    changed = True
                new.append(inst)
            if changed:
                b.instructions = new


def make_in_maps(x, Wq, Wk, Wv, Wo, bo):
    import ml_dtypes
    bf = ml_dtypes.bfloat16
    scale = np.float32(DH) ** np.float32(-0.5)
    # [H, D, DH] -> [D, H*DH]; fold the 1/sqrt(DH) score scale into Wq
    wq_m = np.ascontiguousarray(
        Wq.transpose(1, 0, 2).reshape(D, H * DH) * scale).astype(bf)
    wk_m = np.ascontiguousarray(Wk.transpose(1, 0, 2).reshape(D, H * DH)).astype(bf)
    wv_m = np.ascontiguousarray(Wv.transpose(1, 0, 2).reshape(D, H * DH)).astype(bf)
    wo_m = np.ascontiguousarray(Wo).astype(bf)
    bo_m = np.ascontiguousarray(bo.reshape(1, D)).astype(np.float32)
    return [
        {
            "xt": np.ascontiguousarray(np.asarray(x[b]).T).astype(bf),
            "wq": wq_m, "wk": wk_m, "wv": wv_m, "wo": wo_m, "bo": bo_m,
        }
        for b in range(B)
    ]


_NC_CACHE = []


def kernel(x, Wq, Wk, Wv, Wo, bo):
    from concourse.bass_utils import run_bass_kernel_spmd

    x = np.asarray(x)
    if not _NC_CACHE:
        _NC_CACHE.append(build_nc())
    nc = _NC_CACHE[0]
    in_maps = make_in_maps(x, np.asarray(Wq), np.asarray(Wk), np.asarray(Wv),
                           np.asarray(Wo), np.asarray(bo))
    res = run_bass_kernel_spmd(nc, in_maps, core_ids=list(range(B)))
    return np.stack([res.results[b]["out"] for b in range(B)]).astype(np.float32)


# revision 13
# speedup vs baseline: 1.4548x; 1.0050x over previous
"""Multi-head causal self-attention on 8 Trainium2 NeuronCores.

Problem: B=8, T=1024, D=1024, H=16 heads, DH=64.
    q,k,v = einsum('btd,hdk->bhtk', x, W{q,k,v})
    scores = q @ k.T / sqrt(DH), causal mask, softmax
    out = (softmax @ v) reshaped -> [B,T,H*DH] @ Wo + bo

Sharding: batch-parallel, one batch element per core (B == n_cores == 8).
No collectives; weights replicated to every core.

Per-core dataflow (transpose-free), v2 = bf16 streams + merged ACT work:
  All matmul operands are bf16 (PSUM accumulation stays fp32), which keeps
  the PE at its 1 row/cycle stream rate and halves SBUF/DMA traffic; the
  softmax denominator path stays fp32.
  xT [d,t] (host-transposed) lives with d on partitions; QT/KT come out as
  [dh, t] (heads pair-packed on partitions) and V as [t, dh] (heads
  quad-packed).  Scores are computed transposed, ST[s,q] = KT.T@QT, into a
  single 2-bank PSUM pair tile [128, 2(head), 512] so ONE ACT exp and ONE
  GpSimd affine-select cover both heads of the pair (halves the per-j-step
  instruction overhead on the two engines that pace the attention inner
  loop).  A ones column appended to V yields the softmax denominator in row
  64 of the AV psum.  exp() is applied without max-subtraction (scores are
  O(5) for randn inputs) and causal masking zeroes exp(S) after the fact.
  Normalization is deferred and flushed during the NEXT quad's projection
  phase: per (pair, chunk) the two heads' denominators are packed to a
  [2, 512] tile (DVE copies may shift partitions), reciprocal'd in one ACT
  instruction, broadcast to 128 partitions with a single K=2 selector
  matmul, and applied with one DVE multiply.  The Wo projection consumes
  OT directly as the stationary operand, producing final[q,d] which DMAs
  out contiguously.

This walrus build only allows ONE sync-wait per instruction, so a
post-scheduling pass hoists extra waits onto inserted PE no-ops.
"""

import sys

for _p in ("/opt/trn_rl_repo", "/root/.axon_site/_ro/trn_rl_repo"):
    if _p not in sys.path:
        sys.path.insert(0, _p)

import numpy as np

import concourse.bass as bass
import concourse.mybir as mybir
import concourse.tile as tile

f32 = mybir.dt.float32
f32r = mybir.dt.float32r
bf16 = mybir.dt.bfloat16

B, T, D, H, DH = 8, 1024, 1024, 16, 64
NP = 128            # partitions
NC = 512            # matmul free-dim chunk (PSUM bank = 512 fp32)
KT_ = D // NP       # 8 contraction tiles over d
NT = T // NP        # 8 tiles over t (s and q tiles)
NCH = T // NC       # 2 free-dim chunks over q
NPAIR = H // 2      # 8 head pairs   (QT/KT pack 2 heads on partitions)
NQUAD = H // 4      # 4 head quads   (V packs 4 heads on free dim)


def build_nc(split_waits=True):
    nc = bass.Bass(trn_type="TRN2")
    xt = nc.dram_tensor("xt", [D, T], bf16, kind="ExternalInput")
    wq = nc.dram_tensor("wq", [D, H * DH], bf16, kind="ExternalInput")
    wk = nc.dram_tensor("wk", [D, H * DH], bf16, kind="ExternalInput")
    wv = nc.dram_tensor("wv", [D, H * DH], bf16, kind="ExternalInput")
    wo = nc.dram_tensor("wo", [H * DH, D], bf16, kind="ExternalInput")
    bo = nc.dram_tensor("bo", [1, D], f32, kind="ExternalInput")
    out = nc.dram_tensor("out", [T, D], f32, kind="ExternalOutput")

    with tile.TileContext(nc) as tc:
        _mha(tc, nc, xt, wq, wk, wv, wo, bo, out)

    if split_waits:
        _split_waits(nc)
    return nc


def _mha(tc, nc, xt, wq, wk, wv, wo, bo, out):
    import contextlib

    ctx = contextlib.ExitStack()
    singles = ctx.enter_context(tc.tile_pool(name="singles", bufs=1))
    bigpool = ctx.enter_context(tc.tile_pool(name="bigpool", bufs=1))
    wpool = ctx.enter_context(tc.tile_pool(name="wpool", bufs=2))
    qkpool = ctx.enter_context(tc.tile_pool(name="qkpool", bufs=4))
    vpool = ctx.enter_context(tc.tile_pool(name="vpool", bufs=2))
    pexpool = ctx.enter_context(tc.tile_pool(name="pexpool", bufs=4))
    avpool = ctx.enter_context(tc.tile_pool(name="avpool", bufs=5))
    denpool = ctx.enter_context(tc.tile_pool(name="denpool", bufs=2))
    recpool = ctx.enter_context(tc.tile_pool(name="recpool", bufs=2))
    fpool = ctx.enter_context(tc.tile_pool(name="fpool", bufs=2))
    ps_proj = ctx.enter_context(tc.tile_pool(name="ps_proj", bufs=2, space="PSUM"))
    ps_st = ctx.enter_context(tc.tile_pool(name="ps_st", bufs=2, space="PSUM"))
    ps_av = ctx.enter_context(tc.tile_pool(name="ps_av", bufs=2, space="PSUM"))

    def act_recip(out_ap, in_ap):
        """ACT-engine reciprocal via raw InstActivation (nc.scalar.activation
        refuses Reciprocal; ~1.5e-6 rel err on our denominator range)."""
        ins = [nc.scalar.lower_ap(in_ap)]
        for arg in (0.0, 1.0, 0.0):                     # bias, scale, alpha
            ins.append(mybir.ImmediateValue(dtype=f32, value=arg))
        nc.scalar.add_instruction(mybir.InstActivation(
            name=nc.get_next_instruction_name(),
            func=mybir.ActivationFunctionType.Reciprocal,
            ins=ins,
            outs=[nc.scalar.lower_ap(out_ap)],
        ))

    with ctx:
        # --- resident constants ---------------------------------------------
        onesf = singles.tile([NP, 1], f32)
        nc.vector.memset(onesf, 1.0)
        ones_row = singles.tile([1, DH], f32r)           # K=1 bcast matmul lhsT
        nc.vector.tensor_copy(out=ones_row, in_=onesf[0:1, 0:1].to_broadcast((1, DH)))

        # x^T and Wo share one 2MB slot: Wo is only needed after the last
        # QKV projection has consumed x^T
        xt_sb = bigpool.tile([NP, KT_, T], bf16, tag="big", name="xt_sb")
        xtr = xt.rearrange("(kt p) t -> p kt t", p=NP)
        nc.sync.dma_start(out=xt_sb[:, :, 0:NC], in_=xtr[:, :, 0:NC])
        bo_bc = singles.tile([NP, D], f32)               # bias broadcast to rows

        # out^T accumulator for all heads: [dh(pair-packed), pair, q]
        ot_sb = singles.tile([NP, NPAIR, T], bf16)

        # deferred normalization: all 8 denominator rows of a quad live in one
        # [1, 8(item), NC] tile so a single ACT reciprocal per quad serves the
        # whole flush (Exp<->Reciprocal table reloads cost 1.3us each);
        # flushed inside the NEXT quad's projection phase
        norm_pending = []

        def flush_normalizes():
            items = list(norm_pending)
            norm_pending.clear()
            den_q = items[0][1]
            rec = recpool.tile([1, 8, NC], f32r, tag="rec", name="rec")
            act_recip(rec, den_q)
            for avsb, den, idx, pair, c in items:
                for hh in range(2):
                    bc_ps = ps_proj.tile([DH, NC], f32, tag="proj_ps", name="bc_ps")
                    nc.tensor.matmul(
                        out=bc_ps, lhsT=ones_row,
                        rhs=rec[0:1, idx * 2 + hh, :], start=True, stop=True)
                    nc.vector.tensor_mul(
                        out=ot_sb[hh * DH:(hh + 1) * DH, pair, c * NC:(c + 1) * NC],
                        in0=avsb[hh * DH:(hh + 1) * DH, :],
                        in1=bc_ps,
                    )

        wo_sb_holder = []

        for quad in range(NQUAD):
            cs = quad * 4 * DH                          # column start in w mats
            wq_sb = wpool.tile([NP, KT_, 4 * DH], bf16, tag="wq")
            wk_sb = wpool.tile([NP, KT_, 4 * DH], bf16, tag="wk")
            wv_sb = wpool.tile([NP, KT_, 4 * DH], bf16, tag="wv")
            nc.sync.dma_start(
                out=wq_sb, in_=wq[:, cs:cs + 4 * DH].rearrange("(kt p) c -> p kt c", p=NP))
            nc.sync.dma_start(
                out=wk_sb, in_=wk[:, cs:cs + 4 * DH].rearrange("(kt p) c -> p kt c", p=NP))
            nc.sync.dma_start(
                out=wv_sb, in_=wv[:, cs:cs + 4 * DH].rearrange("(kt p) c -> p kt c", p=NP))
            if quad == 0:
                # second half of x^T and the bias land after quad-0 weights so
                # the first projection matmuls start as early as possible
                nc.sync.dma_start(out=xt_sb[:, :, NC:T], in_=xtr[:, :, NC:T])
                nc.sync.dma_start(out=bo_bc, in_=bo[0:1, :].to_broadcast((NP, D)))

            # per-quad denominator collector (see flush_normalizes)
            den_q = denpool.tile([1, 8, NC], f32, tag="den", name="den")

            # --- QT / KT projections: [2*DH(partitions), T] per head pair ----
            qk_tiles = {}
            for name, w_sb in (("q", wq_sb), ("k", wk_sb)):
                for pp in range(2):                      # pair within quad
                    t_sb = qkpool.tile([NP, T], bf16, tag=f"{name}t", name=f"{name}t_sb")
                    for c in range(NCH):
                        psum = ps_proj.tile([NP, NC], f32, name="proj_ps")
                        for kd in range(KT_):
                            nc.tensor.matmul(
                                out=psum,
                                lhsT=w_sb[:, kd, pp * NP:(pp + 1) * NP],
                                rhs=xt_sb[:, kd, c * NC:(c + 1) * NC],
                                start=(kd == 0), stop=(kd == KT_ - 1),
                            )
                        nc.vector.tensor_copy(out=t_sb[:, c * NC:(c + 1) * NC], in_=psum)
                    qk_tiles[(name, pp)] = t_sb

            # previous quad's softmax normalizations run here, hidden under
            # the projection matmul stream
            if norm_pending:
                flush_normalizes()

            # --- V (+ones col): [t(partitions), head, s-tile, DH+1] ----------
            v1_sb = vpool.tile([NP, 4, NT, DH + 1], bf16)
            nc.vector.tensor_copy(
                out=v1_sb[:, :, :, DH:DH + 1],
                in_=onesf.to_broadcast((NP, 4, NT, 1)))
            for tt in range(NT):
                psum = ps_proj.tile([NP, 4 * DH], f32, name="vproj_ps", tag="proj_ps")
                for kd in range(KT_):
                    nc.tensor.matmul(
                        out=psum,
                        lhsT=xt_sb[:, kd, tt * NP:(tt + 1) * NP],
                        rhs=wv_sb[:, kd, :],
                        start=(kd == 0), stop=(kd == KT_ - 1),
                    )
                for h in range(4):
                    nc.vector.tensor_copy(
                        out=v1_sb[:, h, tt, 0:DH], in_=psum[:, h * DH:(h + 1) * DH])

            if quad == NQUAD - 1:
                # Wo reuses x^T's slot (x^T fully consumed by the V matmuls
                # above); the DMA overlaps this quad's attention phase
                wo_sb = bigpool.tile([NP, KT_, D], bf16, tag="big", name="wo_sb")
                nc.sync.dma_start(
                    out=wo_sb, in_=wo.rearrange("(kt p) d -> p kt d", p=NP))
                wo_sb_holder.append(wo_sb)

            # --- attention: scores+exp+AV pipelined at the s-tile level ------
            # diagonal blocks only compute their live columns (causal trim);
            # AV matmuls for s-tile j-1 are emitted after the score matmuls
            # for s-tile j so PE overlaps ACT's exp / GpSimd's mask-select
            for pp in range(2):
                pair = quad * 2 + pp
                qt = qk_tiles[("q", pp)]
                kt = qk_tiles[("k", pp)]
                for c in range(NCH):
                    jmax = 4 * c + 4                    # causal: s-tiles 0..jmax-1
                    av = [ps_av.tile([DH + 1, NC], f32, name="av_ps", tag="av_ps")
                          for _ in range(2)]

                    def _emit_st(j):
                        co = min(max(0, j - 4 * c) * NP, NC - NP)   # col trim
                        stp = ps_st.tile([NP, 2, NC], f32, name="st_ps", tag="st_ps")
                        for hh in range(2):             # head within pair
                            hp = hh * DH                # partition offset (0|64)
                            nc.tensor.matmul(
                                out=stp[:, hh, co:NC],
                                lhsT=kt[hp:hp + DH, j * NP:(j + 1) * NP],
                                rhs=qt[hp:hp + DH, c * NC + co:(c + 1) * NC],
                                start=True, stop=True,
                            )
                        p_sb = pexpool.tile([NP, 2, NC], bf16, name="p_sb")
                        nc.scalar.activation(
                            out=p_sb[:, :, co:NC], in_=stp[:, :, co:NC],
                            func=mybir.ActivationFunctionType.Exp)
                        if j >= 4 * c:                  # diagonal block: mask
                            nc.gpsimd.affine_select(
                                out=p_sb[:, :, co:NC], in_=p_sb[:, :, co:NC],
                                pattern=[[0, 2], [1, NC - co]],
                                compare_op=mybir.AluOpType.is_ge,
                                fill=0.0,
                                base=c * NC + co - j * NP,
                                channel_multiplier=-1,
                            )
                        return co, p_sb

                    def _emit_av(j, co, p_sb):
                        for hh in range(2):
                            h = 2 * pp + hh             # head within quad
                            nc.tensor.matmul(
                                out=av[hh][0:DH + 1, co:NC],
                                lhsT=v1_sb[:, h, j, :],
                                rhs=p_sb[:, hh, co:NC],
                                start=(j == 0), stop=(j == jmax - 1),
                                skip_group_check=True,
                            )

                    prev = None
                    for j in range(jmax):
                        cur = (j,) + _emit_st(j)
                        if prev is not None:
                            _emit_av(*prev)
                        prev = cur
                    _emit_av(*prev)

                    # drain: AV outputs + denominators (row 64), pair-packed
                    avsb = avpool.tile([NP, NC], f32, name="avsb")
                    idx = pp * NCH + c
                    for hh in range(2):
                        nc.vector.tensor_copy(
                            out=avsb[hh * DH:(hh + 1) * DH, :],
                            in_=av[hh][0:DH, :])
                        nc.vector.tensor_copy(
                            out=den_q[0:1, idx * 2 + hh, :],
                            in_=av[hh][DH:DH + 1, :])
                    norm_pending.append((avsb, den_q, idx, pair, c))

        flush_normalizes()                              # last quad's items
        wo_sb = wo_sb_holder[0]

        # --- Wo projection: final[q, d] = sum_pair OT.T @ Wo + bo ------------
        for qi in range(NT):
            f_sb = fpool.tile([NP, D], f32, name="f_sb")
            for dc in range(NCH):
                wo_ps = ps_av.tile([NP, NC], f32, tag="av_ps", name="wo_ps")
                for pp in range(NPAIR):
                    nc.tensor.matmul(
                        out=wo_ps,
                        lhsT=ot_sb[:, pp, qi * NP:(qi + 1) * NP],
                        rhs=wo_sb[:, pp, dc * NC:(dc + 1) * NC],
                        start=(pp == 0), stop=(pp == NPAIR - 1),
                    )
                nc.vector.tensor_add(
                    out=f_sb[:, dc * NC:(dc + 1) * NC],
                    in0=wo_ps,
                    in1=bo_bc[:, dc * NC:(dc + 1) * NC],
                )
            nc.sync.dma_start(out=out[qi * NP:(qi + 1) * NP, :], in_=f_sb)


def _split_waits(nc, max_waits=1):
    """Walrus on this target allows one sync-wait per instruction; hoist
    extras onto no-ops inserted just before the offending instruction."""
    for f in nc.m.functions:
        for b in f.blocks:
            insts = b.instructions
            new = []
            changed = False
            for inst in insts:
                si = inst.sync_info
                if si is not None and len(si.on_wait) > max_waits:
                    waits = list(si.on_wait)
                    extra, keep = waits[:-max_waits], waits[-max_waits:]
                    for j, w in enumerate(extra):
                        new.append(mybir.InstNoOp(
                            name=f"{inst.name}-wnop{j}",
                            sync_info=mybir.SyncInfo(on_wait=[w], on_update=[]),
                            engine=inst.engine,
                            bass_nofuse=True,
                        ))
                    inst.sync_info = mybir.SyncInfo(
                        on_wait=keep, on_update=list(si.on_update))
                    changed = True
                new.append(inst)
            if changed:
                b.instructions = new


def make_in_maps(x, Wq, Wk, Wv, Wo, bo):
    import ml_dtypes
    bf = ml_dtypes.bfloat16
    scale = np.float32(DH) ** np.float32(-0.5)
    # [H, D, DH] -> [D, H*DH]; fold the 1/sqrt(DH) score scale into Wq
    wq_m = np.ascontiguousarray(
        Wq.transpose(1, 0, 2).reshape(D, H * DH) * scale).astype(bf)
    wk_m = np.ascontiguousarray(Wk.transpose(1, 0, 2).reshape(D, H * DH)).astype(bf)
    wv_m = np.ascontiguousarray(Wv.transpose(1, 0, 2).reshape(D, H * DH)).astype(bf)
    wo_m = np.ascontiguousarray(Wo).astype(bf)
    bo_m = np.ascontiguousarray(bo.reshape(1, D)).astype(np.float32)
    return [
        {
            "xt": np.ascontiguousarray(np.asarray(x[b]).T).astype(bf),
            "wq": wq_m, "wk": wk_m, "wv": wv_m, "wo": wo_m, "bo": bo_m,
        }
        for b in range(B)
    ]


_NC_CACHE = []


def kernel(x, Wq, Wk, Wv, Wo, bo):
    from concourse.bass_utils import run_bass_kernel_spmd

    x = np.asarray(x)
    if not _NC_CACHE:
        _NC_CACHE.append(build_nc())
    nc = _NC_CACHE[0]
    in_maps = make_in_maps(x, np.asarray(Wq), np.asarray(Wk), np.asarray(Wv),
                           np.asarray(Wo), np.asarray(bo))
    res = run_bass_kernel_spmd(nc, in_maps, core_ids=list(range(B)))
    return np.stack([res.results[b]["out"] for b in range(B)]).astype(np.float32)
